# revision 23
# baseline (speedup 1.0000x reference)
"""Bounding-box kernel for Trainium2 (Bass/Tile), 8-core SPMD.

Problem: mask [128, 1, 512, 512] f32 -> bbox [128, 4] int32
  (y_min, x_min, y_max, x_max) of the region where mask >= 0.5,
  with (0, 0, H, W) when a row/col has no hit.

Strategy (per core, 16 images):
  - Stream each image as one [128, 4, 512] DMA (partition p holds rows
    4p..4p+3, contiguous 8KB descriptors). The stream runs at ~420 GB/s
    when nothing stalls the trigger chain, so every per-engine cost must
    stay under the ~2.44 us/image arrival cadence.
  - Threshold on ACT: h = Relu(x*2^34 - (2^33-512)) in bf16, which is
    exactly 0 iff x < 0.5 and >= 512 otherwise (exact for every f32).
    The >=512 scale enables compare-free extents via a min-trick:
       hi_raw = reduce_max(min(mass, idx+1))     (= idx_max+1, or 0)
       lo_raw = reduce_max(min(mass, 512-idx))   (= 512-idx_min, or 0)
  - Column masses: one-hot lhsT matmuls accumulate into PSUM
    (partition = image), split into group A (images 0..12, extents
    computed DURING the stream, own bbox DMA) and group B (13..15,
    short tail chain, 3-row bbox DMA). B gets partition-0-based tiles
    (PSUM reads with a partition offset are rejected by the verifier).
  - Row extents: DVE rowmax per image (bf16 dst) -> [128, 4] slices,
    min-trick against per-partition row-index consts, packed [128,16],
    PE-transposed; A-part early, B-part at the tail.
  - No gpsimd pre-add: PE matmuls pipeline at ~216 ns each, and keeping
    gpsimd/DVE under the cadence is what keeps the DMA stream saturated.
  - Image 15 arrives as two half loads so the final chain is short.
"""

import numpy as np
import ml_dtypes
from contextlib import ExitStack

import concourse.bass as bass
import concourse.bacc as bacc
import concourse.tile as tile
import concourse.mybir as mybir
from concourse.bass_utils import run_bass_kernel_spmd

N_CORES = 8
N, H, W = 128, 512, 512
NPC = N // N_CORES          # images per core = 16
P = 128                     # SBUF partitions
NBLK = H // P               # 4 row blocks per image
F32 = mybir.dt.float32
BF16 = mybir.dt.bfloat16
I32 = mybir.dt.int32

NA = 13                     # images in group A (early extents)
NB = NPC - NA               # images in group B (tail) = 3

# Relu(x * 2^34 - (2^33 - 512)) == 0 iff x < 0.5, >= 512 iff x >= 0.5,
# exact for EVERY f32 x: x*2^34 is exact (power-of-2 scale); for
# x < 0.5, x*2^34 <= 2^33 - 512 so the true sum is <= 0; for x >= 0.5
# the true sum is >= 512 and rounds (f32 then bf16) to >= 512.
ACT_SCALE = float(2**34)
ACT_BIAS = float(512 - 2**33)

TRACE = False               # test.py sets True to capture a HW profile
LAST_RESULTS = None         # BassKernelResults of the last run

USE_TTR = False             # tensor_tensor_reduce crashes the exec unit on HW
FOLD_IMAGES = tuple(range(NPC - 1))    # half-fold on gpsimd; img15 DVE-direct
FP16_BX = True              # B-group X chain in fp16 (2x-mode probe)
X_TT_ON_GP = False          # Pool supports only add/mult TT — min stays on DVE

_compiled = None


def _build_nc():
    nc = bacc.Bacc(
        "TRN2", target_bir_lowering=False, debug=False, num_devices=N_CORES
    )
    mask_d = nc.dram_tensor("mask", [NPC * H, W], F32, kind="ExternalInput").ap()
    # one-hots: A images as 16-wide slices, then B images as 3-wide slices
    oneh_d = nc.dram_tensor(
        "onehot", [P, NPC * NPC + NB * NB], BF16, kind="ExternalInput"
    ).ap()
    # packed f32 consts: ident [0:128] | yconL x16 [128:192] | yconH x16
    pack_d = nc.dram_tensor(
        "cpack", [P, P + 2 * NPC * NBLK], F32, kind="ExternalInput"
    ).ap()
    # packed f32 X consts on 16 partitions: xp1 [0:512] | xm512 [512:1024]
    xcon_d = nc.dram_tensor("xcon", [NPC, 2 * W], F32, kind="ExternalInput").ap()
    bbox_d = nc.dram_tensor("bbox", [NPC, 4], I32, kind="ExternalOutput").ap()

    with tile.TileContext(nc) as tc, ExitStack() as ctx:
        consts = ctx.enter_context(tc.tile_pool(name="consts", bufs=1))
        xpool = ctx.enter_context(tc.tile_pool(name="x", bufs=4))
        hpool = ctx.enter_context(tc.tile_pool(name="h", bufs=3))
        lastpool = ctx.enter_context(tc.tile_pool(name="last", bufs=2))
        small = ctx.enter_context(tc.tile_pool(name="small", bufs=1))
        scratch = ctx.enter_context(tc.tile_pool(name="scratch", bufs=2))
        hfpool = ctx.enter_context(tc.tile_pool(name="hf", bufs=3))
        psum = ctx.enter_context(tc.tile_pool(name="psum", bufs=1, space="PSUM"))

        # consts ride the scalar (ACT HWDGE) queue so the sync queue's
        # mask stream descriptors are issued without delay
        with tc.high_priority():
            oneh = consts.tile([P, NPC * NPC + NB * NB], BF16)
            nc.scalar.dma_start(out=oneh[:], in_=oneh_d)
            cpack = consts.tile([P, P + 2 * NPC * NBLK], F32)
            nc.scalar.dma_start(out=cpack[:], in_=pack_d)
            xcon = consts.tile([NPC, 2 * W], F32)
            nc.scalar.dma_start(out=xcon[:], in_=xcon_d)
            act_bias = consts.tile([P, 1], F32)
            nc.vector.memset(act_bias[:], ACT_BIAS)
        ident = cpack[:, 0:P]
        # [128, 64] image-tiled row consts: col i*4+b = 512-(4p+b) / 4p+b+1
        yrepL = cpack[:, P:P + NPC * NBLK]
        yrepH = cpack[:, P + NPC * NBLK:P + 2 * NPC * NBLK]
        xp1 = xcon[:, 0:W]                       # [16, 512] = x+1
        xm512 = xcon[:, W:2 * W]                 # [16, 512] = 512-x

        # rowmax[p, i*4 + b]: max over x of h for image row r = 4p + b.
        # bf16 dst (max of bf16 values is exact; 2B dst enables DVE 2x).
        rowmax = small.tile([P, NPC * NBLK], BF16)
        rowmax_v = rowmax.rearrange("p (i b) -> p i b", i=NPC)
        # per-image Y min-trick candidates packed [128, 2, 16]
        # ([:, 0, i] = lo cand, [:, 1, i] = hi cand; col = image)
        lohiP = small.tile([P, 2, NPC], F32)
        loP = lohiP[:, 0, :]
        hiP = lohiP[:, 1, :]
        # column-mass PSUM groups
        cntA = psum.tile([NPC, W], F32)   # images 0..NA-1 (rows 13..15 zero)
        cntB = psum.tile([NB, W], F32)    # images NA..15 on partitions 0..2
        # transposed Y candidate stages
        tpsLA = psum.tile([NA, P], F32)
        tpsHA = psum.tile([NA, P], F32)
        tpsLB = psum.tile([NB, P], F32)
        tpsHB = psum.tile([NB, P], F32)

        # raw extents: col0 = By (512-ymin | 0), col1 = Bx, col2 = Ay
        # (ymax+1 | 0), col3 = Ax.  A rows on partitions 0..12; B group
        # has its own partition-0-based tile.
        rawA = small.tile([NA, 4], F32)
        rawB = small.tile([NB, 4], F32)

        def rowmax_of(i, h_img):
            """rowmax for image i. h_img: [P, NBLK, W] bf16.

            For FOLD_IMAGES, gpsimd folds blocks {0,1} 512->256 by ADD
            (sum of non-negative masses stays 0 iff no hit, >=512
            otherwise; Pool has no max TT) while DVE direct-reduces
            blocks {2,3}. Splitting keeps BOTH engines under the ~2.4us
            DMA cadence — a full-image gp fold (2.13us) made the coupled
            ACT->gp->DVE loop settle at ~2.5us/image, stretching the
            ACT-triggered DMA stream itself.
            """
            if i in FOLD_IMAGES:
                hf = hfpool.tile([P, 2, W // 2], BF16, tag="hf")
                nc.gpsimd.tensor_add(
                    hf[:], h_img[:, 0:2, 0:W // 2], h_img[:, 0:2, W // 2:W]
                )
                nc.vector.tensor_reduce(
                    out=rowmax_v[:, i, 2:NBLK], in_=h_img[:, 2:NBLK, :],
                    axis=mybir.AxisListType.X, op=mybir.AluOpType.max,
                )
                nc.vector.tensor_reduce(
                    out=rowmax_v[:, i, 0:2], in_=hf[:],
                    axis=mybir.AxisListType.X, op=mybir.AluOpType.max,
                )
            else:
                nc.vector.tensor_reduce(
                    out=rowmax_v[:, i, :], in_=h_img,
                    axis=mybir.AxisListType.X, op=mybir.AluOpType.max,
                )

        def y_batch(s, n, tag):
            """Y min-trick candidates for images [s, s+n) in one batch.

            Batching kills the per-image small-op traffic on DVE (which
            also suffers SBUF contention with concurrent gpsimd folds).
            """
            k = n * NBLK
            rmf = scratch.tile([P, NPC * NBLK], F32, tag=tag + "rm")
            nc.vector.tensor_copy(
                rmf[:, 0:k], rowmax[:, s * NBLK:(s + n) * NBLK]
            )
            cand = scratch.tile([P, 2, NPC * NBLK], F32, tag=tag + "cd")
            nc.vector.tensor_tensor(
                out=cand[:, 0, 0:k], in0=rmf[:, 0:k],
                in1=yrepL[:, s * NBLK:(s + n) * NBLK],
                op=mybir.AluOpType.min,
            )
            nc.vector.tensor_tensor(
                out=cand[:, 1, 0:k], in0=rmf[:, 0:k],
                in1=yrepH[:, s * NBLK:(s + n) * NBLK],
                op=mybir.AluOpType.min,
            )
            cand_v = cand.rearrange("p s (i b) -> p s i b", b=NBLK)
            nc.vector.tensor_reduce(
                out=lohiP[:, :, s:s + n], in_=cand_v[:, :, 0:n, :],
                axis=mybir.AxisListType.X, op=mybir.AluOpType.max,
            )

        F16 = mybir.dt.float16
        xcon16 = small.tile([NPC, 2 * W], F16)

        def x_chain16(cnt, nrows, raw, tag):
            """B X extents in fp16: ACT casts PSUM->fp16 (saturating to
            inf is fine: min(inf, c) = c), DVE min/reduce run all-2B."""
            c16 = scratch.tile([NB, W], F16, tag=tag + "c")
            nc.scalar.activation(
                c16[0:nrows, :], cnt[0:nrows, :],
                mybir.ActivationFunctionType.Copy,
            )
            cand = scratch.tile([NB, 2 * W], F16, tag=tag + "d")
            nc.vector.tensor_tensor(
                out=cand[0:nrows, 0:W], in0=c16[0:nrows, :],
                in1=xcon16[0:nrows, 0:W], op=mybir.AluOpType.min,
            )
            nc.vector.tensor_tensor(
                out=cand[0:nrows, W:2 * W], in0=c16[0:nrows, :],
                in1=xcon16[0:nrows, W:2 * W], op=mybir.AluOpType.min,
            )
            r16 = scratch.tile([NB, 2], F16, tag=tag + "r")
            cand_v = cand.rearrange("p (s x) -> p s x", s=2)
            nc.vector.tensor_reduce(
                out=r16[0:nrows, :], in_=cand_v[0:nrows, :, :],
                axis=mybir.AxisListType.X, op=mybir.AluOpType.max,
            )
            nc.vector.tensor_copy(raw[0:nrows, 3:4], r16[0:nrows, 0:1])
            nc.vector.tensor_copy(raw[0:nrows, 1:2], r16[0:nrows, 1:2])

        def x_chain(cnt, nrows, raw, tag):
            """Group X extents: cnt [nrows, W] PSUM -> raw cols 1 and 3."""
            tt = nc.gpsimd if X_TT_ON_GP else nc.vector
            candH = scratch.tile([NPC, W], F32, tag=tag)
            tt.tensor_tensor(
                out=candH[0:nrows, :], in0=cnt[0:nrows, :],
                in1=xp1[0:nrows, :], op=mybir.AluOpType.min,
            )
            nc.vector.tensor_reduce(
                out=raw[0:nrows, 3:4], in_=candH[0:nrows, :],
                axis=mybir.AxisListType.X, op=mybir.AluOpType.max,
            )
            candL = scratch.tile([NPC, W], F32, tag=tag)
            tt.tensor_tensor(
                out=candL[0:nrows, :], in0=cnt[0:nrows, :],
                in1=xm512[0:nrows, :], op=mybir.AluOpType.min,
            )
            nc.vector.tensor_reduce(
                out=raw[0:nrows, 1:2], in_=candL[0:nrows, :],
                axis=mybir.AxisListType.X, op=mybir.AluOpType.max,
            )

        def y_finish(s, nrows, tpsL_t, tpsH_t, raw):
            """Transpose packed Y candidates for images [s, s+nrows)."""
            nc.tensor.matmul(
                tpsL_t[:, :], loP[:, s:s + nrows], ident,
                is_transpose=True, start=True, stop=True,
            )
            nc.tensor.matmul(
                tpsH_t[:, :], hiP[:, s:s + nrows], ident,
                is_transpose=True, start=True, stop=True,
            )
            nc.vector.tensor_reduce(
                out=raw[0:nrows, 0:1], in_=tpsL_t[:, :],
                axis=mybir.AxisListType.X, op=mybir.AluOpType.max,
            )
            nc.vector.tensor_reduce(
                out=raw[0:nrows, 2:3], in_=tpsH_t[:, :],
                axis=mybir.AxisListType.X, op=mybir.AluOpType.max,
            )

        def fixup(raw, nrows, tag):
            """raw -> bbox int32 rows (returned tile).

            G = (A_raw > 0) * 512; lo = G - B_raw; hi = A_raw + 512 - G.
            """
            gm = scratch.tile([NPC, 2], F32, tag=tag + "g")
            nc.vector.tensor_scalar(
                gm[0:nrows, :], raw[0:nrows, 2:4], 0.0, float(H),
                mybir.AluOpType.is_gt, mybir.AluOpType.mult,
            )
            bf = scratch.tile([NPC, 4], F32, tag=tag + "f")
            nc.vector.tensor_sub(bf[0:nrows, 0:2], gm[0:nrows, :], raw[0:nrows, 0:2])
            t5 = scratch.tile([NPC, 2], F32, tag=tag + "t")
            nc.vector.tensor_scalar_add(t5[0:nrows, :], raw[0:nrows, 2:4], float(H))
            nc.vector.tensor_sub(bf[0:nrows, 2:4], t5[0:nrows, :], gm[0:nrows, :])
            bi = scratch.tile([NPC, 4], I32, tag=tag + "i")
            nc.vector.tensor_copy(bi[0:nrows, :], bf[0:nrows, :])
            return bi

        # ---- images 0..13 as dual-image DMAs (256 x 8KB descriptors
        # per instruction amortizes per-instruction overhead; the HWDGE
        # descriptor rate ~20ns/desc is the stream floor) ----
        def act_mm_rowmax(i, x_img, h_img):
            """threshold + matmuls + rowmax for one image view."""
            nc.scalar.activation(
                h_img, x_img, mybir.ActivationFunctionType.Relu,
                bias=act_bias[:], scale=ACT_SCALE,
            )
            if i < NA:
                lhsT = oneh[:, i * NPC:(i + 1) * NPC]
                cnt, st = cntA, (i == 0)
            else:
                j = i - NA
                lhsT = oneh[:, NPC * NPC + j * NB:NPC * NPC + (j + 1) * NB]
                cnt, st = cntB, (j == 0)
            for b in range(NBLK):
                nc.tensor.matmul(
                    cnt[:, :], lhsT, h_img[:, b, :],
                    start=(st and b == 0),
                    stop=(i == NA - 1 and b == NBLK - 1),
                )

        for t in range(7):
            x = xpool.tile([P, 2, NBLK, W], F32, tag="x")
            nc.sync.dma_start(
                out=x[:],
                in_=mask_d[2 * t * H:(2 * t + 2) * H, :]
                .rearrange("(i p b) w -> p i b w", i=2, p=P),
            )
            h = hpool.tile([P, 2, NBLK, W], BF16, tag="h")
            for j in range(2):
                i = 2 * t + j
                act_mm_rowmax(i, x[:, j], h[:, j])
                if i == NA:
                    # group A chains run here, during the tail stream
                    # (bbox-A DMA is issued late on the sync queue — an
                    # early issue would head-of-line block mask loads)
                    y_batch(0, NA, "A")
                    if FP16_BX:
                        nc.vector.tensor_copy(xcon16[:], xcon[:])
                    x_chain(cntA, NA, rawA, "xcandA")
                    y_finish(0, NA, tpsLA, tpsHA, rawA)
                    biA = fixup(rawA, NA, "A")
                rowmax_of(i, h[:, j])

        # ---- image 14: single-image DMA ----
        i = NPC - 2
        x14 = lastpool.tile([P, NBLK, W], F32, tag="x14")
        nc.sync.dma_start(
            out=x14[:],
            in_=mask_d[i * H:(i + 1) * H, :].rearrange("(p b) w -> p b w", p=P),
        )
        h14 = lastpool.tile([P, NBLK, W], BF16, tag="h14")
        act_mm_rowmax(i, x14[:], h14[:])
        rowmax_of(i, h14[:])

        # ---- image 15: single DMA, ACT split in halves so MMs and
        # rowmax pipeline behind the two ACT chunks ----
        i = NPC - 1
        j = i - NA
        lhsT = oneh[:, NPC * NPC + j * NB:NPC * NPC + (j + 1) * NB]
        x15 = lastpool.tile([P, NBLK, W], F32, tag="x15")
        nc.sync.dma_start(
            out=x15[:],
            in_=mask_d[i * H:(i + 1) * H, :].rearrange("(p b) w -> p b w", p=P),
        )
        h15 = lastpool.tile([P, NBLK, W], BF16, tag="h15")
        for u in range(2):
            nc.scalar.activation(
                h15[:, 2 * u:2 * u + 2, :], x15[:, 2 * u:2 * u + 2, :],
                mybir.ActivationFunctionType.Relu,
                bias=act_bias[:], scale=ACT_SCALE,
            )
            for b in (2 * u, 2 * u + 1):
                nc.tensor.matmul(
                    cntB[:, :], lhsT, h15[:, b, :],
                    start=False, stop=(b == NBLK - 1),
                )
            nc.vector.tensor_reduce(
                out=rowmax_v[:, i, 2 * u:2 * u + 2],
                in_=h15[:, 2 * u:2 * u + 2, :],
                axis=mybir.AxisListType.X, op=mybir.AluOpType.max,
            )

        # Y candidates for group B (rowmax written; batch of 3 images)
        y_batch(NA, NB, "B")

        # ---- tail: A bbox DMA (overlaps B chain), then group B ----
        nc.sync.dma_start(out=bbox_d[0:NA, :], in_=biA[0:NA, :])
        if FP16_BX:
            x_chain16(cntB, NB, rawB, "xcB16")
        else:
            x_chain(cntB, NB, rawB, "xcandB")
        y_finish(NA, NB, tpsLB, tpsHB, rawB)
        biB = fixup(rawB, NB, "B")
        nc.sync.dma_start(out=bbox_d[NA:NPC, :], in_=biB[0:NB, :])

    nc.compile()
    return nc


def _consts():
    oneh = np.zeros((P, NPC * NPC + NB * NB), dtype=ml_dtypes.bfloat16)
    for i in range(NPC):
        oneh[:, i * NPC + i] = 1.0      # A layout (only 0..NA-1 used)
    for j in range(NB):
        oneh[:, NPC * NPC + j * NB + j] = 1.0   # B layout
    ident = np.eye(P, dtype=np.float32)
    # block b on partition p is image row r = 4p + b
    p = np.arange(P)
    b = np.arange(NBLK)
    r = (NBLK * p[:, None] + b[None, :]).astype(np.float32)  # [128, 4]
    yrepL = np.tile(float(H) - r, (1, NPC)).astype(np.float32)
    yrepH = np.tile(r + 1.0, (1, NPC)).astype(np.float32)
    pack = np.concatenate([ident, yrepL, yrepH], axis=1).astype(np.float32)
    f = np.arange(W, dtype=np.float32)
    xp1 = np.broadcast_to(f + 1.0, (NPC, W))
    xm512 = np.broadcast_to(float(W) - f, (NPC, W))
    xcon = np.concatenate([xp1, xm512], axis=1).astype(np.float32)
    return oneh, pack, xcon


def kernel(mask):
    global _compiled, LAST_RESULTS
    mask = np.ascontiguousarray(np.asarray(mask), dtype=np.float32)
    assert mask.shape == (N, 1, H, W), mask.shape
    if _compiled is None:
        _compiled = _build_nc()
    nc = _compiled
    oneh, pack, xcon = _consts()
    m = mask.reshape(N, H, W)
    in_maps = []
    for c in range(N_CORES):
        in_maps.append({
            "mask": np.ascontiguousarray(
                m[c * NPC:(c + 1) * NPC].reshape(NPC * H, W)
            ),
            "onehot": oneh,
            "cpack": pack,
            "xcon": xcon,
        })
    res = run_bass_kernel_spmd(nc, in_maps, list(range(N_CORES)), trace=TRACE)
    LAST_RESULTS = res
    out = np.concatenate([res.results[c]["bbox"] for c in range(N_CORES)], axis=0)
    return out.astype(np.int32, copy=False)


# revision 24
# speedup vs baseline: 1.1539x; 1.1539x over previous
"""Bounding-box kernel for Trainium2 (Bass/Tile), 8-core SPMD.

Problem: mask [128, 1, 512, 512] f32 -> bbox [128, 4] int32
  (y_min, x_min, y_max, x_max) of the region where mask >= 0.5,
  with (0, 0, H, W) when a row/col has no hit.

Strategy (per core, 16 images):
  - Stream each image as one [128, 4, 512] DMA (partition p holds rows
    4p..4p+3, contiguous 8KB descriptors). The stream runs at ~420 GB/s
    when nothing stalls the trigger chain, so every per-engine cost must
    stay under the ~2.44 us/image arrival cadence.
  - Threshold on ACT: h = Relu(x*2^34 - (2^33-512)) in bf16, which is
    exactly 0 iff x < 0.5 and >= 512 otherwise (exact for every f32).
    The >=512 scale enables compare-free extents via a min-trick:
       hi_raw = reduce_max(min(mass, idx+1))     (= idx_max+1, or 0)
       lo_raw = reduce_max(min(mass, 512-idx))   (= 512-idx_min, or 0)
  - Column masses: one-hot lhsT matmuls accumulate into PSUM
    (partition = image), split into group A (images 0..12, extents
    computed DURING the stream, own bbox DMA) and group B (13..15,
    short tail chain, 3-row bbox DMA). B gets partition-0-based tiles
    (PSUM reads with a partition offset are rejected by the verifier).
  - Row extents: DVE rowmax per image (bf16 dst) -> [128, 4] slices,
    min-trick against per-partition row-index consts, packed [128,16],
    PE-transposed; A-part early, B-part at the tail.
  - No gpsimd pre-add: PE matmuls pipeline at ~216 ns each, and keeping
    gpsimd/DVE under the cadence is what keeps the DMA stream saturated.
  - Image 15 arrives as two half loads so the final chain is short.
"""

import numpy as np
import ml_dtypes
from contextlib import ExitStack

import concourse.bass as bass
import concourse.bacc as bacc
import concourse.tile as tile
import concourse.mybir as mybir
from concourse.bass_utils import run_bass_kernel_spmd

N_CORES = 8
N, H, W = 128, 512, 512
NPC = N // N_CORES          # images per core = 16
P = 128                     # SBUF partitions
NBLK = H // P               # 4 row blocks per image
F32 = mybir.dt.float32
BF16 = mybir.dt.bfloat16
I32 = mybir.dt.int32

NA = 13                     # images in group A (early extents)
NB = NPC - NA               # images in group B (tail) = 3

# Relu(x * 2^34 - (2^33 - 512)) == 0 iff x < 0.5, >= 512 iff x >= 0.5,
# exact for EVERY f32 x: x*2^34 is exact (power-of-2 scale); for
# x < 0.5, x*2^34 <= 2^33 - 512 so the true sum is <= 0; for x >= 0.5
# the true sum is >= 512 and rounds (f32 then bf16) to >= 512.
ACT_SCALE = float(2**34)
ACT_BIAS = float(512 - 2**33)

TRACE = False               # test.py sets True to capture a HW profile
LAST_RESULTS = None         # BassKernelResults of the last run

USE_TTR = False             # tensor_tensor_reduce crashes the exec unit on HW
FOLD_IMAGES = tuple(range(NPC - 1))    # half-fold on gpsimd; img15 DVE-direct
FP16_BX = True              # B-group X chain in fp16 (2x-mode probe)
X_TT_ON_GP = False          # Pool supports only add/mult TT — min stays on DVE

_compiled = None


def _build_nc():
    nc = bacc.Bacc(
        "TRN2", target_bir_lowering=False, debug=False, num_devices=N_CORES
    )
    mask_d = nc.dram_tensor("mask", [NPC * H, W], F32, kind="ExternalInput").ap()
    # one-hots: A images as 16-wide slices, then B images as 3-wide slices
    oneh_d = nc.dram_tensor(
        "onehot", [P, NPC * NPC + NB * NB], BF16, kind="ExternalInput"
    ).ap()
    # packed f32 consts: ident [0:128] | yconL x16 [128:192] | yconH x16
    pack_d = nc.dram_tensor(
        "cpack", [P, P + 2 * NPC * NBLK], F32, kind="ExternalInput"
    ).ap()
    # packed f32 X consts on 16 partitions: xp1 [0:512] | xm512 [512:1024]
    xcon_d = nc.dram_tensor("xcon", [NPC, 2 * W], F32, kind="ExternalInput").ap()
    bbox_d = nc.dram_tensor("bbox", [NPC, 4], I32, kind="ExternalOutput").ap()

    with tile.TileContext(nc) as tc, ExitStack() as ctx:
        consts = ctx.enter_context(tc.tile_pool(name="consts", bufs=1))
        xpool = ctx.enter_context(tc.tile_pool(name="x", bufs=8))
        hpool = ctx.enter_context(tc.tile_pool(name="h", bufs=6))
        lastpool = ctx.enter_context(tc.tile_pool(name="last", bufs=2))
        small = ctx.enter_context(tc.tile_pool(name="small", bufs=1))
        scratch = ctx.enter_context(tc.tile_pool(name="scratch", bufs=2))
        hfpool = ctx.enter_context(tc.tile_pool(name="hf", bufs=3))
        psum = ctx.enter_context(tc.tile_pool(name="psum", bufs=1, space="PSUM"))

        # consts ride the scalar (ACT HWDGE) queue so the sync queue's
        # mask stream descriptors are issued without delay
        with tc.high_priority():
            oneh = consts.tile([P, NPC * NPC + NB * NB], BF16)
            nc.scalar.dma_start(out=oneh[:], in_=oneh_d)
            cpack = consts.tile([P, P + 2 * NPC * NBLK], F32)
            nc.scalar.dma_start(out=cpack[:], in_=pack_d)
            xcon = consts.tile([NPC, 2 * W], F32)
            nc.scalar.dma_start(out=xcon[:], in_=xcon_d)
            act_bias = consts.tile([P, 1], F32)
            nc.vector.memset(act_bias[:], ACT_BIAS)
        ident = cpack[:, 0:P]
        # [128, 64] image-tiled row consts: col i*4+b = 512-(4p+b) / 4p+b+1
        yrepL = cpack[:, P:P + NPC * NBLK]
        yrepH = cpack[:, P + NPC * NBLK:P + 2 * NPC * NBLK]
        xp1 = xcon[:, 0:W]                       # [16, 512] = x+1
        xm512 = xcon[:, W:2 * W]                 # [16, 512] = 512-x

        # rowmax[p, i*4 + b]: max over x of h for image row r = 4p + b.
        # bf16 dst (max of bf16 values is exact; 2B dst enables DVE 2x).
        rowmax = small.tile([P, NPC * NBLK], BF16)
        rowmax_v = rowmax.rearrange("p (i b) -> p i b", i=NPC)
        # per-image Y min-trick candidates packed [128, 2, 16]
        # ([:, 0, i] = lo cand, [:, 1, i] = hi cand; col = image)
        lohiP = small.tile([P, 2, NPC], F32)
        loP = lohiP[:, 0, :]
        hiP = lohiP[:, 1, :]
        # column-mass PSUM groups
        cntA = psum.tile([NPC, W], F32)   # images 0..NA-1 (rows 13..15 zero)
        cntB = psum.tile([NB, W], F32)    # images NA..15 on partitions 0..2
        # transposed Y candidate stages
        tpsLA = psum.tile([NA, P], F32)
        tpsHA = psum.tile([NA, P], F32)
        tpsLB = psum.tile([NB, P], F32)
        tpsHB = psum.tile([NB, P], F32)

        # raw extents: col0 = By (512-ymin | 0), col1 = Bx, col2 = Ay
        # (ymax+1 | 0), col3 = Ax.  A rows on partitions 0..12; B group
        # has its own partition-0-based tile.
        rawA = small.tile([NA, 4], F32)
        rawB = small.tile([NB, 4], F32)

        def rowmax_of(i, h_img):
            """rowmax for image i. h_img: [P, NBLK, W] bf16.

            For FOLD_IMAGES, gpsimd folds blocks {0,1} 512->256 by ADD
            (sum of non-negative masses stays 0 iff no hit, >=512
            otherwise; Pool has no max TT) while DVE direct-reduces
            blocks {2,3}. Splitting keeps BOTH engines under the ~2.4us
            DMA cadence — a full-image gp fold (2.13us) made the coupled
            ACT->gp->DVE loop settle at ~2.5us/image, stretching the
            ACT-triggered DMA stream itself.
            """
            if i in FOLD_IMAGES:
                hf = hfpool.tile([P, 2, W // 2], BF16, tag="hf")
                nc.gpsimd.tensor_add(
                    hf[:], h_img[:, 0:2, 0:W // 2], h_img[:, 0:2, W // 2:W]
                )
                nc.vector.tensor_reduce(
                    out=rowmax_v[:, i, 2:NBLK], in_=h_img[:, 2:NBLK, :],
                    axis=mybir.AxisListType.X, op=mybir.AluOpType.max,
                )
                nc.vector.tensor_reduce(
                    out=rowmax_v[:, i, 0:2], in_=hf[:],
                    axis=mybir.AxisListType.X, op=mybir.AluOpType.max,
                )
            else:
                nc.vector.tensor_reduce(
                    out=rowmax_v[:, i, :], in_=h_img,
                    axis=mybir.AxisListType.X, op=mybir.AluOpType.max,
                )

        def y_batch(s, n, tag):
            """Y min-trick candidates for images [s, s+n) in one batch.

            Batching kills the per-image small-op traffic on DVE (which
            also suffers SBUF contention with concurrent gpsimd folds).
            """
            k = n * NBLK
            rmf = scratch.tile([P, NPC * NBLK], F32, tag=tag + "rm")
            nc.vector.tensor_copy(
                rmf[:, 0:k], rowmax[:, s * NBLK:(s + n) * NBLK]
            )
            cand = scratch.tile([P, 2, NPC * NBLK], F32, tag=tag + "cd")
            nc.vector.tensor_tensor(
                out=cand[:, 0, 0:k], in0=rmf[:, 0:k],
                in1=yrepL[:, s * NBLK:(s + n) * NBLK],
                op=mybir.AluOpType.min,
            )
            nc.vector.tensor_tensor(
                out=cand[:, 1, 0:k], in0=rmf[:, 0:k],
                in1=yrepH[:, s * NBLK:(s + n) * NBLK],
                op=mybir.AluOpType.min,
            )
            cand_v = cand.rearrange("p s (i b) -> p s i b", b=NBLK)
            nc.vector.tensor_reduce(
                out=lohiP[:, :, s:s + n], in_=cand_v[:, :, 0:n, :],
                axis=mybir.AxisListType.X, op=mybir.AluOpType.max,
            )

        F16 = mybir.dt.float16
        xcon16 = small.tile([NPC, 2 * W], F16)

        def x_chain16(cnt, nrows, raw, tag):
            """B X extents in fp16: ACT casts PSUM->fp16 (saturating to
            inf is fine: min(inf, c) = c), DVE min/reduce run all-2B."""
            c16 = scratch.tile([NB, W], F16, tag=tag + "c")
            nc.scalar.activation(
                c16[0:nrows, :], cnt[0:nrows, :],
                mybir.ActivationFunctionType.Copy,
            )
            cand = scratch.tile([NB, 2 * W], F16, tag=tag + "d")
            nc.vector.tensor_tensor(
                out=cand[0:nrows, 0:W], in0=c16[0:nrows, :],
                in1=xcon16[0:nrows, 0:W], op=mybir.AluOpType.min,
            )
            nc.vector.tensor_tensor(
                out=cand[0:nrows, W:2 * W], in0=c16[0:nrows, :],
                in1=xcon16[0:nrows, W:2 * W], op=mybir.AluOpType.min,
            )
            r16 = scratch.tile([NB, 2], F16, tag=tag + "r")
            cand_v = cand.rearrange("p (s x) -> p s x", s=2)
            nc.vector.tensor_reduce(
                out=r16[0:nrows, :], in_=cand_v[0:nrows, :, :],
                axis=mybir.AxisListType.X, op=mybir.AluOpType.max,
            )
            nc.vector.tensor_copy(raw[0:nrows, 3:4], r16[0:nrows, 0:1])
            nc.vector.tensor_copy(raw[0:nrows, 1:2], r16[0:nrows, 1:2])

        def x_chain(cnt, nrows, raw, tag):
            """Group X extents: cnt [nrows, W] PSUM -> raw cols 1 and 3."""
            tt = nc.gpsimd if X_TT_ON_GP else nc.vector
            candH = scratch.tile([NPC, W], F32, tag=tag)
            tt.tensor_tensor(
                out=candH[0:nrows, :], in0=cnt[0:nrows, :],
                in1=xp1[0:nrows, :], op=mybir.AluOpType.min,
            )
            nc.vector.tensor_reduce(
                out=raw[0:nrows, 3:4], in_=candH[0:nrows, :],
                axis=mybir.AxisListType.X, op=mybir.AluOpType.max,
            )
            candL = scratch.tile([NPC, W], F32, tag=tag)
            tt.tensor_tensor(
                out=candL[0:nrows, :], in0=cnt[0:nrows, :],
                in1=xm512[0:nrows, :], op=mybir.AluOpType.min,
            )
            nc.vector.tensor_reduce(
                out=raw[0:nrows, 1:2], in_=candL[0:nrows, :],
                axis=mybir.AxisListType.X, op=mybir.AluOpType.max,
            )

        def y_finish(s, nrows, tpsL_t, tpsH_t, raw):
            """Transpose packed Y candidates for images [s, s+nrows)."""
            nc.tensor.matmul(
                tpsL_t[:, :], loP[:, s:s + nrows], ident,
                is_transpose=True, start=True, stop=True,
            )
            nc.tensor.matmul(
                tpsH_t[:, :], hiP[:, s:s + nrows], ident,
                is_transpose=True, start=True, stop=True,
            )
            nc.vector.tensor_reduce(
                out=raw[0:nrows, 0:1], in_=tpsL_t[:, :],
                axis=mybir.AxisListType.X, op=mybir.AluOpType.max,
            )
            nc.vector.tensor_reduce(
                out=raw[0:nrows, 2:3], in_=tpsH_t[:, :],
                axis=mybir.AxisListType.X, op=mybir.AluOpType.max,
            )

        def fixup(raw, nrows, tag):
            """raw -> bbox int32 rows (returned tile).

            G = (A_raw > 0) * 512; lo = G - B_raw; hi = A_raw + 512 - G.
            """
            gm = scratch.tile([NPC, 2], F32, tag=tag + "g")
            nc.vector.tensor_scalar(
                gm[0:nrows, :], raw[0:nrows, 2:4], 0.0, float(H),
                mybir.AluOpType.is_gt, mybir.AluOpType.mult,
            )
            bf = scratch.tile([NPC, 4], F32, tag=tag + "f")
            nc.vector.tensor_sub(bf[0:nrows, 0:2], gm[0:nrows, :], raw[0:nrows, 0:2])
            t5 = scratch.tile([NPC, 2], F32, tag=tag + "t")
            nc.vector.tensor_scalar_add(t5[0:nrows, :], raw[0:nrows, 2:4], float(H))
            nc.vector.tensor_sub(bf[0:nrows, 2:4], t5[0:nrows, :], gm[0:nrows, :])
            bi = scratch.tile([NPC, 4], I32, tag=tag + "i")
            nc.vector.tensor_copy(bi[0:nrows, :], bf[0:nrows, :])
            return bi

        # ---- images 0..13 as single-image DMAs (128 x 8KB
        # descriptors; the HWDGE descriptor rate ~20ns/desc is the
        # stream floor, and per-image granularity keeps the
        # ACT-triggered ring loop decoupled) ----
        def act_mm_rowmax(i, x_img, h_img):
            """threshold + matmuls for one image view."""
            nc.scalar.activation(
                h_img, x_img, mybir.ActivationFunctionType.Relu,
                bias=act_bias[:], scale=ACT_SCALE,
            )
            if i < NA:
                lhsT = oneh[:, i * NPC:(i + 1) * NPC]
                cnt, st = cntA, (i == 0)
            else:
                j = i - NA
                lhsT = oneh[:, NPC * NPC + j * NB:NPC * NPC + (j + 1) * NB]
                cnt, st = cntB, (j == 0)
            for b in range(NBLK):
                nc.tensor.matmul(
                    cnt[:, :], lhsT, h_img[:, b, :],
                    start=(st and b == 0),
                    stop=(i == NA - 1 and b == NBLK - 1),
                )

        for i in range(NPC - 2):
            x = xpool.tile([P, NBLK, W], F32, tag="x")
            nc.sync.dma_start(
                out=x[:],
                in_=mask_d[i * H:(i + 1) * H, :]
                .rearrange("(p b) w -> p b w", p=P),
            )
            h = hpool.tile([P, NBLK, W], BF16, tag="h")
            act_mm_rowmax(i, x[:], h[:])
            if i == NA:
                # group A chains run here, during the tail stream
                # (bbox-A DMA is issued late on the sync queue — an
                # early issue would head-of-line block mask loads)
                y_batch(0, NA, "A")
                if FP16_BX:
                    nc.vector.tensor_copy(xcon16[:], xcon[:])
                x_chain(cntA, NA, rawA, "xcandA")
                y_finish(0, NA, tpsLA, tpsHA, rawA)
                biA = fixup(rawA, NA, "A")
            rowmax_of(i, h[:])

        # ---- image 14: single-image DMA ----
        i = NPC - 2
        x14 = lastpool.tile([P, NBLK, W], F32, tag="x14")
        nc.sync.dma_start(
            out=x14[:],
            in_=mask_d[i * H:(i + 1) * H, :].rearrange("(p b) w -> p b w", p=P),
        )
        h14 = lastpool.tile([P, NBLK, W], BF16, tag="h14")
        act_mm_rowmax(i, x14[:], h14[:])
        rowmax_of(i, h14[:])

        # ---- image 15: single DMA, ACT split in halves so MMs and
        # rowmax pipeline behind the two ACT chunks ----
        i = NPC - 1
        j = i - NA
        lhsT = oneh[:, NPC * NPC + j * NB:NPC * NPC + (j + 1) * NB]
        x15 = lastpool.tile([P, NBLK, W], F32, tag="x15")
        nc.sync.dma_start(
            out=x15[:],
            in_=mask_d[i * H:(i + 1) * H, :].rearrange("(p b) w -> p b w", p=P),
        )
        h15 = lastpool.tile([P, NBLK, W], BF16, tag="h15")
        for u in range(2):
            nc.scalar.activation(
                h15[:, 2 * u:2 * u + 2, :], x15[:, 2 * u:2 * u + 2, :],
                mybir.ActivationFunctionType.Relu,
                bias=act_bias[:], scale=ACT_SCALE,
            )
            for b in (2 * u, 2 * u + 1):
                nc.tensor.matmul(
                    cntB[:, :], lhsT, h15[:, b, :],
                    start=False, stop=(b == NBLK - 1),
                )
            nc.vector.tensor_reduce(
                out=rowmax_v[:, i, 2 * u:2 * u + 2],
                in_=h15[:, 2 * u:2 * u + 2, :],
                axis=mybir.AxisListType.X, op=mybir.AluOpType.max,
            )

        # Y candidates for group B (rowmax written; batch of 3 images)
        y_batch(NA, NB, "B")

        # ---- tail: A bbox DMA (overlaps B chain), then group B ----
        nc.sync.dma_start(out=bbox_d[0:NA, :], in_=biA[0:NA, :])
        if FP16_BX:
            x_chain16(cntB, NB, rawB, "xcB16")
        else:
            x_chain(cntB, NB, rawB, "xcandB")
        y_finish(NA, NB, tpsLB, tpsHB, rawB)
        biB = fixup(rawB, NB, "B")
        nc.sync.dma_start(out=bbox_d[NA:NPC, :], in_=biB[0:NB, :])

    nc.compile()
    return nc


def _consts():
    oneh = np.zeros((P, NPC * NPC + NB * NB), dtype=ml_dtypes.bfloat16)
    for i in range(NPC):
        oneh[:, i * NPC + i] = 1.0      # A layout (only 0..NA-1 used)
    for j in range(NB):
        oneh[:, NPC * NPC + j * NB + j] = 1.0   # B layout
    ident = np.eye(P, dtype=np.float32)
    # block b on partition p is image row r = 4p + b
    p = np.arange(P)
    b = np.arange(NBLK)
    r = (NBLK * p[:, None] + b[None, :]).astype(np.float32)  # [128, 4]
    yrepL = np.tile(float(H) - r, (1, NPC)).astype(np.float32)
    yrepH = np.tile(r + 1.0, (1, NPC)).astype(np.float32)
    pack = np.concatenate([ident, yrepL, yrepH], axis=1).astype(np.float32)
    f = np.arange(W, dtype=np.float32)
    xp1 = np.broadcast_to(f + 1.0, (NPC, W))
    xm512 = np.broadcast_to(float(W) - f, (NPC, W))
    xcon = np.concatenate([xp1, xm512], axis=1).astype(np.float32)
    return oneh, pack, xcon


def kernel(mask):
    global _compiled, LAST_RESULTS
    mask = np.ascontiguousarray(np.asarray(mask), dtype=np.float32)
    assert mask.shape == (N, 1, H, W), mask.shape
    if _compiled is None:
        _compiled = _build_nc()
    nc = _compiled
    oneh, pack, xcon = _consts()
    m = mask.reshape(N, H, W)
    in_maps = []
    for c in range(N_CORES):
        in_maps.append({
            "mask": np.ascontiguousarray(
                m[c * NPC:(c + 1) * NPC].reshape(NPC * H, W)
            ),
            "onehot": oneh,
            "cpack": pack,
            "xcon": xcon,
        })
    res = run_bass_kernel_spmd(nc, in_maps, list(range(N_CORES)), trace=TRACE)
    LAST_RESULTS = res
    out = np.concatenate([res.results[c]["bbox"] for c in range(N_CORES)], axis=0)
    return out.astype(np.int32, copy=False)


# revision 25
# speedup vs baseline: 1.1542x; 1.0002x over previous
"""Bounding-box kernel for Trainium2 (Bass/Tile), 8-core SPMD.

Problem: mask [128, 1, 512, 512] f32 -> bbox [128, 4] int32
  (y_min, x_min, y_max, x_max) of the region where mask >= 0.5,
  with (0, 0, H, W) when a row/col has no hit.

Strategy (per core, 16 images):
  - Stream each image as one [128, 4, 512] DMA (partition p holds rows
    4p..4p+3, contiguous 8KB descriptors). The stream runs at ~420 GB/s
    when nothing stalls the trigger chain, so every per-engine cost must
    stay under the ~2.44 us/image arrival cadence.
  - Threshold on ACT: h = Relu(x*2^34 - (2^33-512)) in bf16, which is
    exactly 0 iff x < 0.5 and >= 512 otherwise (exact for every f32).
    The >=512 scale enables compare-free extents via a min-trick:
       hi_raw = reduce_max(min(mass, idx+1))     (= idx_max+1, or 0)
       lo_raw = reduce_max(min(mass, 512-idx))   (= 512-idx_min, or 0)
  - Column masses: one-hot lhsT matmuls accumulate into PSUM
    (partition = image), split into group A (images 0..12, extents
    computed DURING the stream, own bbox DMA) and group B (13..15,
    short tail chain, 3-row bbox DMA). B gets partition-0-based tiles
    (PSUM reads with a partition offset are rejected by the verifier).
  - Row extents: DVE rowmax per image (bf16 dst) -> [128, 4] slices,
    min-trick against per-partition row-index consts, packed [128,16],
    PE-transposed; A-part early, B-part at the tail.
  - No gpsimd pre-add: PE matmuls pipeline at ~216 ns each, and keeping
    gpsimd/DVE under the cadence is what keeps the DMA stream saturated.
  - Image 15 arrives as two half loads so the final chain is short.
"""

import numpy as np
import ml_dtypes
from contextlib import ExitStack

import concourse.bass as bass
import concourse.bacc as bacc
import concourse.tile as tile
import concourse.mybir as mybir
from concourse.bass_utils import run_bass_kernel_spmd

N_CORES = 8
N, H, W = 128, 512, 512
NPC = N // N_CORES          # images per core = 16
P = 128                     # SBUF partitions
NBLK = H // P               # 4 row blocks per image
F32 = mybir.dt.float32
BF16 = mybir.dt.bfloat16
I32 = mybir.dt.int32

NA = 13                     # images in group A (early extents)
NB = NPC - NA               # images in group B (tail) = 3

# Relu(x * 2^34 - (2^33 - 512)) == 0 iff x < 0.5, >= 512 iff x >= 0.5,
# exact for EVERY f32 x: x*2^34 is exact (power-of-2 scale); for
# x < 0.5, x*2^34 <= 2^33 - 512 so the true sum is <= 0; for x >= 0.5
# the true sum is >= 512 and rounds (f32 then bf16) to >= 512.
ACT_SCALE = float(2**34)
ACT_BIAS = float(512 - 2**33)

TRACE = False               # test.py sets True to capture a HW profile
LAST_RESULTS = None         # BassKernelResults of the last run

USE_TTR = False             # tensor_tensor_reduce crashes the exec unit on HW
FOLD_IMAGES = tuple(range(NPC - 1))    # half-fold on gpsimd; img15 DVE-direct
FP16_BX = True              # B-group X chain in fp16 (2x-mode probe)
X_TT_ON_GP = False          # Pool supports only add/mult TT — min stays on DVE

_compiled = None


def _build_nc():
    nc = bacc.Bacc(
        "TRN2", target_bir_lowering=False, debug=False, num_devices=N_CORES
    )
    mask_d = nc.dram_tensor("mask", [NPC * H, W], F32, kind="ExternalInput").ap()
    # one-hots: A images as 16-wide slices, then B images as 3-wide slices
    oneh_d = nc.dram_tensor(
        "onehot", [P, NPC * NPC + NB * NB], BF16, kind="ExternalInput"
    ).ap()
    # packed f32 consts: ident [0:128] | yconL x16 [128:192] | yconH x16
    pack_d = nc.dram_tensor(
        "cpack", [P, P + 2 * NPC * NBLK], F32, kind="ExternalInput"
    ).ap()
    # packed f32 X consts on 16 partitions: xp1 [0:512] | xm512 [512:1024]
    xcon_d = nc.dram_tensor("xcon", [NPC, 2 * W], F32, kind="ExternalInput").ap()
    bbox_d = nc.dram_tensor("bbox", [NPC, 4], I32, kind="ExternalOutput").ap()

    with tile.TileContext(nc) as tc, ExitStack() as ctx:
        consts = ctx.enter_context(tc.tile_pool(name="consts", bufs=1))
        xpool = ctx.enter_context(tc.tile_pool(name="x", bufs=8))
        hpool = ctx.enter_context(tc.tile_pool(name="h", bufs=6))
        lastpool = ctx.enter_context(tc.tile_pool(name="last", bufs=2))
        small = ctx.enter_context(tc.tile_pool(name="small", bufs=1))
        scratch = ctx.enter_context(tc.tile_pool(name="scratch", bufs=2))
        hfpool = ctx.enter_context(tc.tile_pool(name="hf", bufs=3))
        psum = ctx.enter_context(tc.tile_pool(name="psum", bufs=1, space="PSUM"))

        # consts ride the scalar (ACT HWDGE) queue so the sync queue's
        # mask stream descriptors are issued without delay
        with tc.high_priority():
            oneh = consts.tile([P, NPC * NPC + NB * NB], BF16)
            nc.scalar.dma_start(out=oneh[:], in_=oneh_d)
            cpack = consts.tile([P, P + 2 * NPC * NBLK], F32)
            nc.scalar.dma_start(out=cpack[:], in_=pack_d)
            xcon = consts.tile([NPC, 2 * W], F32)
            nc.scalar.dma_start(out=xcon[:], in_=xcon_d)
            act_bias = consts.tile([P, 1], F32)
            nc.vector.memset(act_bias[:], ACT_BIAS)
        ident = cpack[:, 0:P]
        # [128, 64] image-tiled row consts: col i*4+b = 512-(4p+b) / 4p+b+1
        yrepL = cpack[:, P:P + NPC * NBLK]
        yrepH = cpack[:, P + NPC * NBLK:P + 2 * NPC * NBLK]
        xp1 = xcon[:, 0:W]                       # [16, 512] = x+1
        xm512 = xcon[:, W:2 * W]                 # [16, 512] = 512-x

        # rowmax[p, i*4 + b]: max over x of h for image row r = 4p + b.
        # bf16 dst (max of bf16 values is exact; 2B dst enables DVE 2x).
        rowmax = small.tile([P, NPC * NBLK], BF16)
        rowmax_v = rowmax.rearrange("p (i b) -> p i b", i=NPC)
        # per-image Y min-trick candidates packed [128, 2, 16]
        # ([:, 0, i] = lo cand, [:, 1, i] = hi cand; col = image)
        lohiP = small.tile([P, 2, NPC], F32)
        loP = lohiP[:, 0, :]
        hiP = lohiP[:, 1, :]
        # column-mass PSUM groups
        cntA = psum.tile([NPC, W], F32)   # images 0..NA-1 (rows 13..15 zero)
        cntB = psum.tile([NB, W], F32)    # images NA..15 on partitions 0..2
        # transposed Y candidate stages
        tpsLA = psum.tile([NA, P], F32)
        tpsHA = psum.tile([NA, P], F32)
        tpsLB = psum.tile([NB, P], F32)
        tpsHB = psum.tile([NB, P], F32)

        # raw extents: col0 = By (512-ymin | 0), col1 = Bx, col2 = Ay
        # (ymax+1 | 0), col3 = Ax.  A rows on partitions 0..12; B group
        # has its own partition-0-based tile.
        rawA = small.tile([NA, 4], F32)
        rawB = small.tile([NB, 4], F32)

        def rowmax_of(i, h_img):
            """rowmax for image i. h_img: [P, NBLK, W] bf16.

            For FOLD_IMAGES, gpsimd folds blocks {0,1} 512->256 by ADD
            (sum of non-negative masses stays 0 iff no hit, >=512
            otherwise; Pool has no max TT) while DVE direct-reduces
            blocks {2,3}. Splitting keeps BOTH engines under the ~2.4us
            DMA cadence — a full-image gp fold (2.13us) made the coupled
            ACT->gp->DVE loop settle at ~2.5us/image, stretching the
            ACT-triggered DMA stream itself.
            """
            if i in FOLD_IMAGES:
                hf = hfpool.tile([P, 2, W // 2], BF16, tag="hf")
                nc.gpsimd.tensor_add(
                    hf[:], h_img[:, 0:2, 0:W // 2], h_img[:, 0:2, W // 2:W]
                )
                nc.vector.tensor_reduce(
                    out=rowmax_v[:, i, 2:NBLK], in_=h_img[:, 2:NBLK, :],
                    axis=mybir.AxisListType.X, op=mybir.AluOpType.max,
                )
                nc.vector.tensor_reduce(
                    out=rowmax_v[:, i, 0:2], in_=hf[:],
                    axis=mybir.AxisListType.X, op=mybir.AluOpType.max,
                )
            else:
                nc.vector.tensor_reduce(
                    out=rowmax_v[:, i, :], in_=h_img,
                    axis=mybir.AxisListType.X, op=mybir.AluOpType.max,
                )

        def y_batch(s, n, tag):
            """Y min-trick candidates for images [s, s+n) in one batch.

            Batching kills the per-image small-op traffic on DVE (which
            also suffers SBUF contention with concurrent gpsimd folds).
            """
            k = n * NBLK
            rmf = scratch.tile([P, NPC * NBLK], F32, tag=tag + "rm")
            nc.vector.tensor_copy(
                rmf[:, 0:k], rowmax[:, s * NBLK:(s + n) * NBLK]
            )
            cand = scratch.tile([P, 2, NPC * NBLK], F32, tag=tag + "cd")
            nc.vector.tensor_tensor(
                out=cand[:, 0, 0:k], in0=rmf[:, 0:k],
                in1=yrepL[:, s * NBLK:(s + n) * NBLK],
                op=mybir.AluOpType.min,
            )
            nc.vector.tensor_tensor(
                out=cand[:, 1, 0:k], in0=rmf[:, 0:k],
                in1=yrepH[:, s * NBLK:(s + n) * NBLK],
                op=mybir.AluOpType.min,
            )
            cand_v = cand.rearrange("p s (i b) -> p s i b", b=NBLK)
            nc.vector.tensor_reduce(
                out=lohiP[:, :, s:s + n], in_=cand_v[:, :, 0:n, :],
                axis=mybir.AxisListType.X, op=mybir.AluOpType.max,
            )

        F16 = mybir.dt.float16
        xcon16 = small.tile([NPC, 2 * W], F16)

        def x_chain16(cnt, nrows, raw, tag):
            """B X extents in fp16: ACT casts PSUM->fp16 (saturating to
            inf is fine: min(inf, c) = c), DVE min/reduce run all-2B."""
            c16 = scratch.tile([NB, W], F16, tag=tag + "c")
            nc.scalar.activation(
                c16[0:nrows, :], cnt[0:nrows, :],
                mybir.ActivationFunctionType.Copy,
            )
            cand = scratch.tile([NB, 2 * W], F16, tag=tag + "d")
            nc.vector.tensor_tensor(
                out=cand[0:nrows, 0:W], in0=c16[0:nrows, :],
                in1=xcon16[0:nrows, 0:W], op=mybir.AluOpType.min,
            )
            nc.vector.tensor_tensor(
                out=cand[0:nrows, W:2 * W], in0=c16[0:nrows, :],
                in1=xcon16[0:nrows, W:2 * W], op=mybir.AluOpType.min,
            )
            r16 = scratch.tile([NB, 2], F16, tag=tag + "r")
            cand_v = cand.rearrange("p (s x) -> p s x", s=2)
            nc.vector.tensor_reduce(
                out=r16[0:nrows, :], in_=cand_v[0:nrows, :, :],
                axis=mybir.AxisListType.X, op=mybir.AluOpType.max,
            )
            nc.vector.tensor_copy(raw[0:nrows, 3:4], r16[0:nrows, 0:1])
            nc.vector.tensor_copy(raw[0:nrows, 1:2], r16[0:nrows, 1:2])

        def x_chain(cnt, nrows, raw, tag):
            """Group X extents: cnt [nrows, W] PSUM -> raw cols 1 and 3."""
            tt = nc.gpsimd if X_TT_ON_GP else nc.vector
            candH = scratch.tile([NPC, W], F32, tag=tag)
            tt.tensor_tensor(
                out=candH[0:nrows, :], in0=cnt[0:nrows, :],
                in1=xp1[0:nrows, :], op=mybir.AluOpType.min,
            )
            nc.vector.tensor_reduce(
                out=raw[0:nrows, 3:4], in_=candH[0:nrows, :],
                axis=mybir.AxisListType.X, op=mybir.AluOpType.max,
            )
            candL = scratch.tile([NPC, W], F32, tag=tag)
            tt.tensor_tensor(
                out=candL[0:nrows, :], in0=cnt[0:nrows, :],
                in1=xm512[0:nrows, :], op=mybir.AluOpType.min,
            )
            nc.vector.tensor_reduce(
                out=raw[0:nrows, 1:2], in_=candL[0:nrows, :],
                axis=mybir.AxisListType.X, op=mybir.AluOpType.max,
            )

        def y_finish(s, nrows, tpsL_t, tpsH_t, raw):
            """Transpose packed Y candidates for images [s, s+nrows)."""
            nc.tensor.matmul(
                tpsL_t[:, :], loP[:, s:s + nrows], ident,
                is_transpose=True, start=True, stop=True,
            )
            nc.tensor.matmul(
                tpsH_t[:, :], hiP[:, s:s + nrows], ident,
                is_transpose=True, start=True, stop=True,
            )
            nc.vector.tensor_reduce(
                out=raw[0:nrows, 0:1], in_=tpsL_t[:, :],
                axis=mybir.AxisListType.X, op=mybir.AluOpType.max,
            )
            nc.vector.tensor_reduce(
                out=raw[0:nrows, 2:3], in_=tpsH_t[:, :],
                axis=mybir.AxisListType.X, op=mybir.AluOpType.max,
            )

        def fixup(raw, nrows, tag):
            """raw -> bbox int32 rows (returned tile).

            G = (A_raw > 0) * 512; lo = G - B_raw; hi = A_raw + 512 - G.
            """
            gm = scratch.tile([NPC, 2], F32, tag=tag + "g")
            nc.vector.tensor_scalar(
                gm[0:nrows, :], raw[0:nrows, 2:4], 0.0, float(H),
                mybir.AluOpType.is_gt, mybir.AluOpType.mult,
            )
            bf = scratch.tile([NPC, 4], F32, tag=tag + "f")
            nc.vector.tensor_sub(bf[0:nrows, 0:2], gm[0:nrows, :], raw[0:nrows, 0:2])
            t5 = scratch.tile([NPC, 2], F32, tag=tag + "t")
            nc.vector.tensor_scalar_add(t5[0:nrows, :], raw[0:nrows, 2:4], float(H))
            nc.vector.tensor_sub(bf[0:nrows, 2:4], t5[0:nrows, :], gm[0:nrows, :])
            bi = scratch.tile([NPC, 4], I32, tag=tag + "i")
            nc.vector.tensor_copy(bi[0:nrows, :], bf[0:nrows, :])
            return bi

        # ---- images 0..13 as single-image DMAs (128 x 8KB
        # descriptors; the HWDGE descriptor rate ~20ns/desc is the
        # stream floor, and per-image granularity keeps the
        # ACT-triggered ring loop decoupled) ----
        def act_mm_rowmax(i, x_img, h_img):
            """threshold + matmuls for one image view."""
            nc.scalar.activation(
                h_img, x_img, mybir.ActivationFunctionType.Relu,
                bias=act_bias[:], scale=ACT_SCALE,
            )
            if i < NA:
                lhsT = oneh[:, i * NPC:(i + 1) * NPC]
                cnt, st = cntA, (i == 0)
            else:
                j = i - NA
                lhsT = oneh[:, NPC * NPC + j * NB:NPC * NPC + (j + 1) * NB]
                cnt, st = cntB, (j == 0)
            for b in range(NBLK):
                nc.tensor.matmul(
                    cnt[:, :], lhsT, h_img[:, b, :],
                    start=(st and b == 0),
                    stop=(i == NA - 1 and b == NBLK - 1),
                )

        for i in range(NPC - 2):
            x = xpool.tile([P, NBLK, W], F32, tag="x")
            nc.sync.dma_start(
                out=x[:],
                in_=mask_d[i * H:(i + 1) * H, :]
                .rearrange("(p b) w -> p b w", p=P),
            )
            h = hpool.tile([P, NBLK, W], BF16, tag="h")
            act_mm_rowmax(i, x[:], h[:])
            if i == NA:
                # group A chains run here, during the tail stream
                # (bbox-A DMA is issued late on the sync queue — an
                # early issue would head-of-line block mask loads)
                y_batch(0, NA, "A")
                if FP16_BX:
                    nc.vector.tensor_copy(xcon16[:], xcon[:])
                x_chain(cntA, NA, rawA, "xcandA")
                y_finish(0, NA, tpsLA, tpsHA, rawA)
                biA = fixup(rawA, NA, "A")
            rowmax_of(i, h[:])

        # ---- image 14: single-image DMA ----
        i = NPC - 2
        x14 = lastpool.tile([P, NBLK, W], F32, tag="x14")
        nc.sync.dma_start(
            out=x14[:],
            in_=mask_d[i * H:(i + 1) * H, :].rearrange("(p b) w -> p b w", p=P),
        )
        h14 = lastpool.tile([P, NBLK, W], BF16, tag="h14")
        act_mm_rowmax(i, x14[:], h14[:])
        rowmax_of(i, h14[:])

        # ---- image 15: single DMA, ACT split in halves so MMs and
        # rowmax pipeline behind the two ACT chunks ----
        i = NPC - 1
        j = i - NA
        lhsT = oneh[:, NPC * NPC + j * NB:NPC * NPC + (j + 1) * NB]
        x15 = lastpool.tile([P, NBLK, W], F32, tag="x15")
        nc.sync.dma_start(
            out=x15[:],
            in_=mask_d[i * H:(i + 1) * H, :].rearrange("(p b) w -> p b w", p=P),
        )
        h15 = lastpool.tile([P, NBLK, W], BF16, tag="h15")
        for u in range(2):
            nc.scalar.activation(
                h15[:, 2 * u:2 * u + 2, :], x15[:, 2 * u:2 * u + 2, :],
                mybir.ActivationFunctionType.Relu,
                bias=act_bias[:], scale=ACT_SCALE,
            )
            for b in (2 * u, 2 * u + 1):
                nc.tensor.matmul(
                    cntB[:, :], lhsT, h15[:, b, :],
                    start=False, stop=(b == NBLK - 1),
                )
            # gpsimd (idle at the tail) folds each half so DVE only
            # pays a 256-wide reduce on the critical path
            hfu = hfpool.tile([P, 2, W // 2], BF16, tag="hf15")
            nc.gpsimd.tensor_add(
                hfu[:], h15[:, 2 * u:2 * u + 2, 0:W // 2],
                h15[:, 2 * u:2 * u + 2, W // 2:W],
            )
            nc.vector.tensor_reduce(
                out=rowmax_v[:, i, 2 * u:2 * u + 2], in_=hfu[:],
                axis=mybir.AxisListType.X, op=mybir.AluOpType.max,
            )

        # Y candidates for group B (rowmax written; batch of 3 images)
        y_batch(NA, NB, "B")

        # ---- tail: A bbox DMA (overlaps B chain), then group B ----
        nc.sync.dma_start(out=bbox_d[0:NA, :], in_=biA[0:NA, :])
        if FP16_BX:
            x_chain16(cntB, NB, rawB, "xcB16")
        else:
            x_chain(cntB, NB, rawB, "xcandB")
        y_finish(NA, NB, tpsLB, tpsHB, rawB)
        biB = fixup(rawB, NB, "B")
        nc.sync.dma_start(out=bbox_d[NA:NPC, :], in_=biB[0:NB, :])

    nc.compile()
    return nc


def _consts():
    oneh = np.zeros((P, NPC * NPC + NB * NB), dtype=ml_dtypes.bfloat16)
    for i in range(NPC):
        oneh[:, i * NPC + i] = 1.0      # A layout (only 0..NA-1 used)
    for j in range(NB):
        oneh[:, NPC * NPC + j * NB + j] = 1.0   # B layout
    ident = np.eye(P, dtype=np.float32)
    # block b on partition p is image row r = 4p + b
    p = np.arange(P)
    b = np.arange(NBLK)
    r = (NBLK * p[:, None] + b[None, :]).astype(np.float32)  # [128, 4]
    yrepL = np.tile(float(H) - r, (1, NPC)).astype(np.float32)
    yrepH = np.tile(r + 1.0, (1, NPC)).astype(np.float32)
    pack = np.concatenate([ident, yrepL, yrepH], axis=1).astype(np.float32)
    f = np.arange(W, dtype=np.float32)
    xp1 = np.broadcast_to(f + 1.0, (NPC, W))
    xm512 = np.broadcast_to(float(W) - f, (NPC, W))
    xcon = np.concatenate([xp1, xm512], axis=1).astype(np.float32)
    return oneh, pack, xcon


def kernel(mask):
    global _compiled, LAST_RESULTS
    mask = np.ascontiguousarray(np.asarray(mask), dtype=np.float32)
    assert mask.shape == (N, 1, H, W), mask.shape
    if _compiled is None:
        _compiled = _build_nc()
    nc = _compiled
    oneh, pack, xcon = _consts()
    m = mask.reshape(N, H, W)
    in_maps = []
    for c in range(N_CORES):
        in_maps.append({
            "mask": np.ascontiguousarray(
                m[c * NPC:(c + 1) * NPC].reshape(NPC * H, W)
            ),
            "onehot": oneh,
            "cpack": pack,
            "xcon": xcon,
        })
    res = run_bass_kernel_spmd(nc, in_maps, list(range(N_CORES)), trace=TRACE)
    LAST_RESULTS = res
    out = np.concatenate([res.results[c]["bbox"] for c in range(N_CORES)], axis=0)
    return out.astype(np.int32, copy=False)
